# revision 32
# baseline (speedup 1.0000x reference)
"""Multi-head attention (nn_MultiHeadAttention) Bass kernel for 8 trn2 cores.

Reference semantics (bug preserved: one shared Wk/bk projects q, k AND v):
    xh = x @ Wk.T + bk            -> [B,S,H,D] for x in {q,k,v}
    scores = qh @ kh.T / sqrt(E)  -> softmax over keys (no max-subtraction
                                     needed: |scores| < ~0.5 for these inputs)
    ctx = attn @ vh
    out = concat(ctx) @ Wo.T + bo

Sharding: heads across cores. Core c owns heads {2c, 2c+1} for both batches
(d_global slice [128c, 128c+128)). Each core projects all tokens onto its
128 projection rows, runs attention for its 4 (batch, head) pairs, and emits
a partial output projection out_c = ctx_c @ Wo[:, slice].T. Host sums the 8
partials (the all-reduce of the sharding hint, done on host) and adds the
closed-form bias terms: since softmax rows sum to 1, the v-bias contributes
exactly Wo @ bk, so the device never applies bk to v.

Device layouts (per core, per batch):
    qhT/khT  [128, 2048] bf16   d_local on partitions, token-major free dim
    vh_aug   [128, 16, 130] bf16  tokens on partitions per 128-block; per
             block: cols 0:64 head A, col 64 = 1.0, 65:129 head B, col 129 =
             1.0. The ones-column makes the ctx matmul emit the softmax
             denominator as psum row 64 for free.
    scores   computed TRANSPOSED st[k, q] so that k (the softmax reduction
             axis) is the PE contraction dim of the ctx matmul. The two
             heads' score matmuls are row-packed (K=64 each at array rows
             0-63 / 64-127) so they run concurrently on the PE.

Emission order pipelines batches: proj(b0), attn(b0), proj(b1), attn(b1) —
attention b0 (ACT-bound) overlaps with projection b1 (PE/DMA-bound).
"""

import os
import sys

import numpy as np

sys.path.insert(0, "/opt/trn_rl_repo")

import ml_dtypes

import concourse.bacc as bacc
import concourse.mybir as mybir
import concourse.tile as tile
from concourse.bass_utils import run_bass_kernel_spmd

B, S, E, H, D = 2, 2048, 1024, 16, 64
BT = B * S  # 4096 tokens total
N_CORES = 8
HPC = H // N_CORES  # 2 heads per core
DL = HPC * D  # 128 projection rows per core
SCALE = 1.0 / float(np.sqrt(np.float32(E)))  # 1/32
FP32 = mybir.dt.float32
BF16 = mybir.dt.bfloat16
NPBF16 = ml_dtypes.bfloat16

# partial outputs in bf16: halves the output DMA and doubles the DVE
# psum->sbuf drain rate; adds ~1e-3 absmax-relative error (verified on HW)
OUT_BF16 = True
OUT_DT = BF16 if OUT_BF16 else FP32

EC = E // 128  # 8 contraction chunks for projections
QC = 512  # query chunk
NKB = S // 128  # 16 key blocks per batch

_CACHE = {}


def _build_nc(repeat=1):
    nc = bacc.Bacc("TRN2", target_bir_lowering=False, debug=False)

    qT = nc.dram_tensor("qT", [E, BT], BF16, kind="ExternalInput")
    kT = nc.dram_tensor("kT", [E, BT], BF16, kind="ExternalInput")
    vT = nc.dram_tensor("vT", [E, BT], BF16, kind="ExternalInput")
    wkT = nc.dram_tensor("wkT", [E, DL], BF16, kind="ExternalInput")
    bk = nc.dram_tensor("bk", [DL, 1], FP32, kind="ExternalInput")
    woT = nc.dram_tensor("woT", [DL, E], BF16, kind="ExternalInput")
    outT = nc.dram_tensor("outT", [E, BT], OUT_DT, kind="ExternalOutput")

    with tile.TileContext(nc) as tc:
        for _ in range(repeat):
            _emit(nc, tc, qT, kT, vT, wkT, bk, woT, outT)
    nc.compile()
    return nc


def _proj_batch(nc, pools, b, qT, kT, vT, qhT, khT, vh_aug):
    """Project batch b's tokens: v -> token-major blocks (no bias, emitted
    first so attention's ctx matmuls unblock early); k, q -> d-major bf16
    with bias added on the DVE."""
    xin, prps, vps_pool, wk_sb, bk_sb = pools
    base = b * S

    # k and q: d-major [128, S]
    for x_dram, dst in ((kT, khT), (qT, qhT)):
        for tt in range(S // 1024):
            ps_pair = [
                prps.tile([128, 512], FP32, tag="pj", name=f"pr{i}") for i in range(2)
            ]
            for ec in range(EC):
                xc = xin.tile([128, 1024], BF16, tag="x")
                nc.sync.dma_start(
                    xc[:],
                    x_dram[
                        ec * 128 : (ec + 1) * 128,
                        base + tt * 1024 : base + (tt + 1) * 1024,
                    ],
                )
                for i in range(2):
                    nc.tensor.matmul(
                        ps_pair[i][:],
                        wk_sb[:, ec, :],
                        xc[:, i * 512 : (i + 1) * 512],
                        start=(ec == 0),
                        stop=(ec == EC - 1),
                    )
            for i in range(2):
                nc.vector.tensor_scalar_add(
                    dst[:, tt * 1024 + i * 512 : tt * 1024 + (i + 1) * 512],
                    ps_pair[i][:],
                    bk_sb[:],
                )


    # v: keep all 8 e-chunks of a 1024-token stripe resident, accumulate one
    # 128-token block at a time (few psum banks live).
    for tt in range(S // 1024):
        chunks = []
        for ec in range(EC):
            xc = xin.tile([128, 1024], BF16, tag="xv", name=f"xv{ec}")
            nc.sync.dma_start(
                xc[:],
                vT[
                    ec * 128 : (ec + 1) * 128,
                    base + tt * 1024 : base + (tt + 1) * 1024,
                ],
            )
            chunks.append(xc)
        for ti in range(8):
            vps = vps_pool.tile([128, 512], FP32, tag="pj", name="vps")[:, 0:DL]
            for ec in range(EC):
                nc.tensor.matmul(
                    vps[:],
                    chunks[ec][:, ti * 128 : (ti + 1) * 128],
                    wk_sb[:, ec, :],
                    start=(ec == 0),
                    stop=(ec == EC - 1),
                )
            blk = tt * 8 + ti
            # one strided copy: psum [128,(2,64)] -> cols {0:64, 65:129}
            dst = vh_aug[:, blk, :].rearrange("p (g c) -> p g c", g=2)[:, :, 0:D]
            src = vps[:].rearrange("p (g c) -> p g c", c=D)
            nc.vector.tensor_copy(dst, src)


def _attn_batch(nc, pools, b, qhT, khT, vh_aug, ctxT, wo_sb, outT, pjps):
    stps, ctxps, expsb, smsb, osb = pools
    kph = os.environ.get("KPHASE", "full")
    noctx = kph == "attn_noctx"
    noout = kph == "attn_noout"

    def emit_norm(qc, ctx_ps):
        qbase = qc * QC
        for h in range(HPC):
            hp = h * D
            recip = smsb.tile([1, QC], FP32, tag="recip")
            nc.vector.reciprocal(recip[:], ctx_ps[h][D : D + 1, :])
            bcast = smsb.tile([D, QC], FP32, tag="bcast")
            nc.gpsimd.partition_broadcast(bcast[:], recip[:])
            nc.vector.tensor_tensor(
                ctxT[hp : hp + D, qbase : qbase + QC],
                ctx_ps[h][0:D, :],
                bcast[:],
                mybir.AluOpType.mult,
            )

    def emit_outproj(qc):
        # drains through the (idle during attention) projection psum slots
        qbase = qc * QC
        for et in range(E // 128):
            ops = pjps.tile([128, QC], FP32, tag="pj", name="ops")
            nc.tensor.matmul(
                ops[:],
                wo_sb[:, et * 128 : (et + 1) * 128],
                ctxT[:, qbase : qbase + QC],
            )
            ot = osb.tile([128, QC], OUT_DT, tag="ot")
            nc.vector.tensor_copy(ot[:], ops[:])
            nc.sync.dma_start(
                outT[et * 128 : (et + 1) * 128, b * S + qbase : b * S + qbase + QC],
                ot[:],
            )

    pending_tail = None
    for qc in range(S // QC):
        qbase = qc * QC
        ctx_ps = [
            ctxps.tile([D + 1, QC], FP32, tag="ctx", name=f"ctx{h}")
            for h in range(HPC)
        ]

        def emit_ctx(kbp, exs):
            # ctx matmuls for k-block pair kbp (run one pair behind st/exp so
            # the in-order PE stream never waits on a just-issued exp)
            for h in range(HPC):
                for j in range(2):
                    kb = kbp * 2 + j
                    nc.tensor.matmul(
                        ctx_ps[h][:],
                        vh_aug[:, kb, h * (D + 1) : h * (D + 1) + D + 1],
                        exs[h][:, j, :],
                        start=(kb == 0),
                        stop=(kb == NKB - 1),
                    )

        from collections import deque

        pend = deque()  # (kbp, [exA, exB]), ctx runs CTX_LAG k-pairs behind
        CTX_LAG = int(os.environ.get("CTX_LAG", "2"))
        for kbp in range(NKB // 2):
            exs = []
            for h in range(HPC):
                hp = h * D  # partition offset of this head
                st = stps.tile([128, 2, QC], FP32, tag="st")
                for j in range(2):
                    kbase = (kbp * 2 + j) * 128
                    nc.tensor.matmul(
                        st[:, j, :],
                        khT[hp : hp + D, kbase : kbase + 128],
                        qhT[hp : hp + D, qbase : qbase + QC],
                        tile_position=(hp, 0),
                    )
                ex = expsb.tile([128, 2, QC], BF16, tag=f"exp{h}")
                nc.scalar.activation(
                    ex[:], st[:], mybir.ActivationFunctionType.Exp, scale=SCALE
                )
                exs.append(ex)
            if not noctx:
                pend.append((kbp, exs))
                if len(pend) > CTX_LAG:
                    emit_ctx(*pend.popleft())
        if not noctx:
            while pend:
                emit_ctx(*pend.popleft())

        if not (noctx or noout):
            emit_norm(qc, ctx_ps)
            if pending_tail is not None:
                emit_outproj(pending_tail)
            pending_tail = qc
    if pending_tail is not None:
        emit_outproj(pending_tail)


def _emit(nc, tc, qT, kT, vT, wkT, bk, woT, outT):
    from contextlib import ExitStack

    with ExitStack() as ctx:
        persist = ctx.enter_context(tc.tile_pool(name="persist", bufs=1))
        wk_sb = persist.tile([128, EC, DL], BF16, tag="wk")
        nc.sync.dma_start(wk_sb[:], wkT.rearrange("(ec p) d -> p ec d", p=128))
        wo_sb = persist.tile([128, E], BF16, tag="wo")
        nc.sync.dma_start(wo_sb[:], woT[:])
        bk_sb = persist.tile([DL, 1], FP32, tag="bk")
        nc.sync.dma_start(bk_sb[:], bk[:])
        ones_sb = persist.tile([1, D], FP32, tag="ones")
        nc.vector.memset(ones_sb[:], 1.0)
        nc.ones_sb = ones_sb

        qhT = [persist.tile([128, S], BF16, tag="qhT", name=f"qhT{b}") for b in range(B)]
        khT = [persist.tile([128, S], BF16, tag="khT", name=f"khT{b}") for b in range(B)]
        vh_aug = [
            persist.tile([128, S // 128, 2 * D + 2], BF16, tag="vh", name=f"vh{b}")
            for b in range(B)
        ]
        ctxT = [persist.tile([128, S], BF16, tag="ctxT", name=f"ctxT{b}") for b in range(B)]
        for b in range(B):
            nc.vector.memset(vh_aug[b][:], 1.0)
        if os.environ.get("KPHASE", "").startswith("attn"):
            for b in range(B):
                nc.vector.memset(qhT[b][:], 0.0)
                nc.vector.memset(khT[b][:], 0.0)

        # deep input prefetch: "x" slots idle during attention, so batch b+1's
        # chunks stream in while batch b's attention runs
        xin = ctx.enter_context(tc.tile_pool(name="xin", bufs=20))
        expsb = ctx.enter_context(
            tc.tile_pool(name="expsb", bufs=int(os.environ.get("EXP_BUFS", "6")))
        )
        smsb = ctx.enter_context(tc.tile_pool(name="smsb", bufs=4))
        osb = ctx.enter_context(tc.tile_pool(name="osb", bufs=int(os.environ.get("OSB_BUFS", "8"))))

        # PSUM static budget (8 banks): proj 2 + st 2x2 + ctx 2x1 = 8. The
        # single shared proj pool stays open across batches so proj(b1)'s
        # matmuls overlap attention(b0).
        pjps = ctx.enter_context(tc.tile_pool(name="pjps", bufs=2, space="PSUM"))
        stps = ctx.enter_context(tc.tile_pool(name="stps", bufs=2, space="PSUM"))
        ctxps = ctx.enter_context(tc.tile_pool(name="ctxps", bufs=2, space="PSUM"))

        phase = os.environ.get("KPHASE", "full")
        if phase in ("attn_noctx", "attn_noout"):
            phase = "attn"
        phase = {"attn": "attn", "proj": "proj", "full": "full"}[phase]
        for b in range(B):
            if phase in ("full", "proj"):
                proj_pools = (xin, pjps, pjps, wk_sb, bk_sb)
                _proj_batch(nc, proj_pools, b, qT, kT, vT, qhT[b], khT[b], vh_aug[b])
            if phase in ("full", "attn"):
                attn_pools = (stps, ctxps, expsb, smsb, osb)
                _attn_batch(
                    nc, attn_pools, b, qhT[b], khT[b], vh_aug[b], ctxT[b], wo_sb,
                    outT, pjps
                )


def _get_nc():
    if "nc" not in _CACHE:
        _CACHE["nc"] = _build_nc()
    return _CACHE["nc"]


def _prep_in_maps(q, k, v, Wk, bk, Wo):
    qT = np.ascontiguousarray(q.reshape(BT, E).T.astype(NPBF16))
    kT = np.ascontiguousarray(k.reshape(BT, E).T.astype(NPBF16))
    vT = np.ascontiguousarray(v.reshape(BT, E).T.astype(NPBF16))

    in_maps = []
    for c in range(N_CORES):
        sl = slice(DL * c, DL * (c + 1))
        in_maps.append(
            {
                "qT": qT,
                "kT": kT,
                "vT": vT,
                "wkT": np.ascontiguousarray(Wk[sl, :].T.astype(NPBF16)),
                "bk": np.ascontiguousarray(bk[sl].reshape(DL, 1)),
                "woT": np.ascontiguousarray(Wo[:, sl].T.astype(NPBF16)),
            }
        )
    return in_maps


def _unshard(results, Wk, bk, Wo, bo):
    outT = np.zeros((E, BT), dtype=np.float64)
    for r in results:
        outT += r["outT"].astype(np.float64)
    out = outT.T.reshape(B, S, E)
    out += (Wo.astype(np.float64) @ bk.astype(np.float64) + bo.astype(np.float64))[
        None, None, :
    ]
    return out.astype(np.float32)


def kernel(q, k, v, Wk, bk, Wo, bo):
    q = np.asarray(q, dtype=np.float32)
    k = np.asarray(k, dtype=np.float32)
    v = np.asarray(v, dtype=np.float32)
    Wk = np.asarray(Wk, dtype=np.float32)
    bk = np.asarray(bk, dtype=np.float32)
    Wo = np.asarray(Wo, dtype=np.float32)
    bo = np.asarray(bo, dtype=np.float32)

    in_maps = _prep_in_maps(q, k, v, Wk, bk, Wo)
    nc = _get_nc()
    trace = bool(int(os.environ.get("KERNEL_TRACE", "0")))
    res = run_bass_kernel_spmd(nc, in_maps, core_ids=list(range(N_CORES)), trace=trace)
    _CACHE["last_results"] = res
    return _unshard(res.results, Wk, bk, Wo, bo)


# revision 33
# speedup vs baseline: 1.1397x; 1.1397x over previous
"""Multi-head attention (nn_MultiHeadAttention) Bass kernel for 8 trn2 cores.

Reference semantics (bug preserved: one shared Wk/bk projects q, k AND v):
    xh = x @ Wk.T + bk            -> [B,S,H,D] for x in {q,k,v}
    scores = qh @ kh.T / sqrt(E)  -> softmax over keys (no max-subtraction
                                     needed: |scores| < ~0.5 for these inputs)
    ctx = attn @ vh
    out = concat(ctx) @ Wo.T + bo

Sharding: heads across cores. Core c owns heads {2c, 2c+1} for both batches
(d_global slice [128c, 128c+128)). Each core projects all tokens onto its
128 projection rows, runs attention for its 4 (batch, head) pairs, and emits
a partial output projection out_c = ctx_c @ Wo[:, slice].T. Host sums the 8
partials (the all-reduce of the sharding hint, done on host) and adds the
closed-form bias terms: since softmax rows sum to 1, the v-bias contributes
exactly Wo @ bk, so the device never applies bk to v.

Device layouts (per core, per batch):
    qhT/khT  [128, 2048] bf16   d_local on partitions, token-major free dim
    vh_aug   [128, 16, 130] bf16  tokens on partitions per 128-block; per
             block: cols 0:64 head A, col 64 = 1.0, 65:129 head B, col 129 =
             1.0. The ones-column makes the ctx matmul emit the softmax
             denominator as psum row 64 for free.
    scores   computed TRANSPOSED st[k, q] so that k (the softmax reduction
             axis) is the PE contraction dim of the ctx matmul. The two
             heads' score matmuls are row-packed (K=64 each at array rows
             0-63 / 64-127) so they run concurrently on the PE.

Emission order pipelines batches: proj(b0), attn(b0), proj(b1), attn(b1) —
attention b0 (ACT-bound) overlaps with projection b1 (PE/DMA-bound).
"""

import os
import sys

import numpy as np

sys.path.insert(0, "/opt/trn_rl_repo")

import ml_dtypes

import concourse.bacc as bacc
import concourse.mybir as mybir
import concourse.tile as tile
from concourse.bass_utils import run_bass_kernel_spmd

B, S, E, H, D = 2, 2048, 1024, 16, 64
BT = B * S  # 4096 tokens total
N_CORES = 8
HPC = H // N_CORES  # 2 heads per core
DL = HPC * D  # 128 projection rows per core
SCALE = 1.0 / float(np.sqrt(np.float32(E)))  # 1/32
FP32 = mybir.dt.float32
BF16 = mybir.dt.bfloat16
NPBF16 = ml_dtypes.bfloat16

# partial outputs in bf16: halves the output DMA and doubles the DVE
# psum->sbuf drain rate; adds ~1e-3 absmax-relative error (verified on HW)
OUT_BF16 = True
OUT_DT = BF16 if OUT_BF16 else FP32

EC = E // 128  # 8 contraction chunks for projections
QC = 512  # query chunk
NKB = S // 128  # 16 key blocks per batch

_CACHE = {}


def _build_nc(repeat=1):
    nc = bacc.Bacc("TRN2", target_bir_lowering=False, debug=False)

    qT = nc.dram_tensor("qT", [E, BT], BF16, kind="ExternalInput")
    kT = nc.dram_tensor("kT", [E, BT], BF16, kind="ExternalInput")
    vT = nc.dram_tensor("vT", [E, BT], BF16, kind="ExternalInput")
    wkT = nc.dram_tensor("wkT", [E, DL], BF16, kind="ExternalInput")
    bk = nc.dram_tensor("bk", [DL, 1], FP32, kind="ExternalInput")
    woT = nc.dram_tensor("woT", [DL, E], BF16, kind="ExternalInput")
    outT = nc.dram_tensor("outT", [E, BT], OUT_DT, kind="ExternalOutput")

    with tile.TileContext(nc) as tc:
        for _ in range(repeat):
            _emit(nc, tc, qT, kT, vT, wkT, bk, woT, outT)
    nc.compile()
    return nc


def _proj_batch(nc, pools, b, qT, kT, vT, qhT, khT, vh_aug):
    """Project batch b's tokens: v -> token-major blocks (no bias, emitted
    first so attention's ctx matmuls unblock early); k, q -> d-major bf16
    with bias added on the DVE."""
    xin, prps, vps_pool, wk_sb, bk_sb = pools
    base = b * S

    # k and q: d-major [128, S]
    for x_dram, dst in ((kT, khT), (qT, qhT)):
        for tt in range(S // 1024):
            ps_pair = [
                prps.tile([128, 512], FP32, tag="pj", name=f"pr{i}") for i in range(2)
            ]
            for ec in range(EC):
                xc = xin.tile([128, 1024], BF16, tag="x")
                nc.sync.dma_start(
                    xc[:],
                    x_dram[
                        ec * 128 : (ec + 1) * 128,
                        base + tt * 1024 : base + (tt + 1) * 1024,
                    ],
                )
                for i in range(2):
                    nc.tensor.matmul(
                        ps_pair[i][:],
                        wk_sb[:, ec, :],
                        xc[:, i * 512 : (i + 1) * 512],
                        start=(ec == 0),
                        stop=(ec == EC - 1),
                    )
            for i in range(2):
                nc.vector.tensor_scalar_add(
                    dst[:, tt * 1024 + i * 512 : tt * 1024 + (i + 1) * 512],
                    ps_pair[i][:],
                    bk_sb[:],
                )


    # v: keep all 8 e-chunks of a 1024-token stripe resident, accumulate one
    # 128-token block at a time (few psum banks live).
    for tt in range(S // 1024):
        chunks = []
        for ec in range(EC):
            xc = xin.tile([128, 1024], BF16, tag="xv", name=f"xv{ec}")
            nc.sync.dma_start(
                xc[:],
                vT[
                    ec * 128 : (ec + 1) * 128,
                    base + tt * 1024 : base + (tt + 1) * 1024,
                ],
            )
            chunks.append(xc)
        for ti in range(8):
            vps = vps_pool.tile([128, 512], FP32, tag="pj", name="vps")[:, 0:DL]
            for ec in range(EC):
                nc.tensor.matmul(
                    vps[:],
                    chunks[ec][:, ti * 128 : (ti + 1) * 128],
                    wk_sb[:, ec, :],
                    start=(ec == 0),
                    stop=(ec == EC - 1),
                )
            blk = tt * 8 + ti
            # one strided copy: psum [128,(2,64)] -> cols {0:64, 65:129}
            dst = vh_aug[:, blk, :].rearrange("p (g c) -> p g c", g=2)[:, :, 0:D]
            src = vps[:].rearrange("p (g c) -> p g c", c=D)
            nc.vector.tensor_copy(dst, src)


def _attn_batch(nc, pools, b, qhT, khT, vh_aug, ctxT, wo_sb, outT, pjps):
    stps, ctxps, expsb, smsb, osb = pools
    kph = os.environ.get("KPHASE", "full")
    noctx = kph == "attn_noctx"
    noout = kph == "attn_noout"

    def emit_norm(qc, ctx_ps):
        qbase = qc * QC
        for h in range(HPC):
            hp = h * D
            recip = smsb.tile([1, QC], FP32, tag="recip")
            nc.vector.reciprocal(recip[:], ctx_ps[h][D : D + 1, :])
            bcast = smsb.tile([D, QC], FP32, tag="bcast")
            nc.gpsimd.partition_broadcast(bcast[:], recip[:])
            nc.vector.tensor_tensor(
                ctxT[hp : hp + D, qbase : qbase + QC],
                ctx_ps[h][0:D, :],
                bcast[:],
                mybir.AluOpType.mult,
            )

    def emit_outproj(qc):
        # drains through the (idle during attention) projection psum slots
        qbase = qc * QC
        for et in range(E // 128):
            ops = pjps.tile([128, QC], FP32, tag="pj", name="ops")
            nc.tensor.matmul(
                ops[:],
                wo_sb[:, et * 128 : (et + 1) * 128],
                ctxT[:, qbase : qbase + QC],
            )
            ot = osb.tile([128, QC], OUT_DT, tag="ot")
            nc.vector.tensor_copy(ot[:], ops[:])
            nc.sync.dma_start(
                outT[et * 128 : (et + 1) * 128, b * S + qbase : b * S + qbase + QC],
                ot[:],
            )

    pending_tail = None
    for qc in range(S // QC):
        qbase = qc * QC
        ctx_ps = [
            ctxps.tile([D + 1, QC], FP32, tag="ctx", name=f"ctx{h}")
            for h in range(HPC)
        ]

        def emit_ctx(kbp, exs):
            # ctx matmuls for k-block pair kbp (run one pair behind st/exp so
            # the in-order PE stream never waits on a just-issued exp)
            for h in range(HPC):
                for j in range(2):
                    kb = kbp * 2 + j
                    nc.tensor.matmul(
                        ctx_ps[h][:],
                        vh_aug[:, kb, h * (D + 1) : h * (D + 1) + D + 1],
                        exs[h][:, j, :],
                        start=(kb == 0),
                        stop=(kb == NKB - 1),
                    )

        from collections import deque

        pend = deque()  # (kbp, [exA, exB]), ctx runs CTX_LAG k-pairs behind
        CTX_LAG = int(os.environ.get("CTX_LAG", "1"))
        for kbp in range(NKB // 2):
            exs = []
            for h in range(HPC):
                hp = h * D  # partition offset of this head
                st = stps.tile([128, 2, QC], FP32, tag="st")
                for j in range(2):
                    kbase = (kbp * 2 + j) * 128
                    nc.tensor.matmul(
                        st[:, j, :],
                        khT[hp : hp + D, kbase : kbase + 128],
                        qhT[hp : hp + D, qbase : qbase + QC],
                        tile_position=(hp, 0),
                    )
                ex = expsb.tile([128, 2, QC], BF16, tag=f"exp{h}")
                nc.scalar.activation(
                    ex[:], st[:], mybir.ActivationFunctionType.Exp, scale=SCALE
                )
                exs.append(ex)
            if not noctx:
                pend.append((kbp, exs))
                if len(pend) > CTX_LAG:
                    emit_ctx(*pend.popleft())
        if not noctx:
            while pend:
                emit_ctx(*pend.popleft())

        if not (noctx or noout):
            emit_norm(qc, ctx_ps)
            if pending_tail is not None:
                emit_outproj(pending_tail)
            pending_tail = qc
    if pending_tail is not None:
        emit_outproj(pending_tail)


def _emit(nc, tc, qT, kT, vT, wkT, bk, woT, outT):
    from contextlib import ExitStack

    with ExitStack() as ctx:
        persist = ctx.enter_context(tc.tile_pool(name="persist", bufs=1))
        wk_sb = persist.tile([128, EC, DL], BF16, tag="wk")
        nc.sync.dma_start(wk_sb[:], wkT.rearrange("(ec p) d -> p ec d", p=128))
        wo_sb = persist.tile([128, E], BF16, tag="wo")
        nc.sync.dma_start(wo_sb[:], woT[:])
        bk_sb = persist.tile([DL, 1], FP32, tag="bk")
        nc.sync.dma_start(bk_sb[:], bk[:])
        ones_sb = persist.tile([1, D], FP32, tag="ones")
        nc.vector.memset(ones_sb[:], 1.0)
        nc.ones_sb = ones_sb

        qhT = [persist.tile([128, S], BF16, tag="qhT", name=f"qhT{b}") for b in range(B)]
        khT = [persist.tile([128, S], BF16, tag="khT", name=f"khT{b}") for b in range(B)]
        vh_aug = [
            persist.tile([128, S // 128, 2 * D + 2], BF16, tag="vh", name=f"vh{b}")
            for b in range(B)
        ]
        ctxT = [persist.tile([128, S], BF16, tag="ctxT", name=f"ctxT{b}") for b in range(B)]
        for b in range(B):
            nc.vector.memset(vh_aug[b][:], 1.0)
        if os.environ.get("KPHASE", "").startswith("attn"):
            for b in range(B):
                nc.vector.memset(qhT[b][:], 0.0)
                nc.vector.memset(khT[b][:], 0.0)

        # deep input prefetch: "x" slots idle during attention, so batch b+1's
        # chunks stream in while batch b's attention runs
        xin = ctx.enter_context(tc.tile_pool(name="xin", bufs=20))
        expsb = ctx.enter_context(
            tc.tile_pool(name="expsb", bufs=int(os.environ.get("EXP_BUFS", "6")))
        )
        smsb = ctx.enter_context(tc.tile_pool(name="smsb", bufs=4))
        osb = ctx.enter_context(tc.tile_pool(name="osb", bufs=int(os.environ.get("OSB_BUFS", "8"))))

        # PSUM static budget (8 banks): proj 2 + st 2x2 + ctx 2x1 = 8. The
        # single shared proj pool stays open across batches so proj(b1)'s
        # matmuls overlap attention(b0).
        pjps = ctx.enter_context(tc.tile_pool(name="pjps", bufs=2, space="PSUM"))
        stps = ctx.enter_context(tc.tile_pool(name="stps", bufs=2, space="PSUM"))
        ctxps = ctx.enter_context(tc.tile_pool(name="ctxps", bufs=2, space="PSUM"))

        phase = os.environ.get("KPHASE", "full")
        if phase in ("attn_noctx", "attn_noout"):
            phase = "attn"
        phase = {"attn": "attn", "proj": "proj", "full": "full"}[phase]
        for b in range(B):
            if phase in ("full", "proj"):
                proj_pools = (xin, pjps, pjps, wk_sb, bk_sb)
                _proj_batch(nc, proj_pools, b, qT, kT, vT, qhT[b], khT[b], vh_aug[b])
            if phase in ("full", "attn"):
                attn_pools = (stps, ctxps, expsb, smsb, osb)
                _attn_batch(
                    nc, attn_pools, b, qhT[b], khT[b], vh_aug[b], ctxT[b], wo_sb,
                    outT, pjps
                )


def _get_nc():
    if "nc" not in _CACHE:
        _CACHE["nc"] = _build_nc()
    return _CACHE["nc"]


def _prep_in_maps(q, k, v, Wk, bk, Wo):
    qT = np.ascontiguousarray(q.reshape(BT, E).T.astype(NPBF16))
    kT = np.ascontiguousarray(k.reshape(BT, E).T.astype(NPBF16))
    vT = np.ascontiguousarray(v.reshape(BT, E).T.astype(NPBF16))

    in_maps = []
    for c in range(N_CORES):
        sl = slice(DL * c, DL * (c + 1))
        in_maps.append(
            {
                "qT": qT,
                "kT": kT,
                "vT": vT,
                "wkT": np.ascontiguousarray(Wk[sl, :].T.astype(NPBF16)),
                "bk": np.ascontiguousarray(bk[sl].reshape(DL, 1)),
                "woT": np.ascontiguousarray(Wo[:, sl].T.astype(NPBF16)),
            }
        )
    return in_maps


def _unshard(results, Wk, bk, Wo, bo):
    outT = np.zeros((E, BT), dtype=np.float64)
    for r in results:
        outT += r["outT"].astype(np.float64)
    out = outT.T.reshape(B, S, E)
    out += (Wo.astype(np.float64) @ bk.astype(np.float64) + bo.astype(np.float64))[
        None, None, :
    ]
    return out.astype(np.float32)


def kernel(q, k, v, Wk, bk, Wo, bo):
    q = np.asarray(q, dtype=np.float32)
    k = np.asarray(k, dtype=np.float32)
    v = np.asarray(v, dtype=np.float32)
    Wk = np.asarray(Wk, dtype=np.float32)
    bk = np.asarray(bk, dtype=np.float32)
    Wo = np.asarray(Wo, dtype=np.float32)
    bo = np.asarray(bo, dtype=np.float32)

    in_maps = _prep_in_maps(q, k, v, Wk, bk, Wo)
    nc = _get_nc()
    trace = bool(int(os.environ.get("KERNEL_TRACE", "0")))
    res = run_bass_kernel_spmd(nc, in_maps, core_ids=list(range(N_CORES)), trace=trace)
    _CACHE["last_results"] = res
    return _unshard(res.results, Wk, bk, Wo, bo)
